# revision 4
# baseline (speedup 1.0000x reference)
"""Trainium2 Bass kernel for CompositeTransitionNet (MoE + KV-memory + 2x gated MLP + vocab adapter).

Sharding: expert-parallel MoE (1 expert/core, host-side top-2 dispatch/combine),
data-parallel over B for KV attention, gated MLPs and the vocab adapter.
All activations kept feature-major ([feature, token]) on device; every weight is
pre-transposed on the host into LHS-T block layout so no device transposes are
needed outside the attention probs.
"""
import os
import sys

for _p in ("/opt/trn_rl_repo", "/root/.axon_site/_ro/trn_rl_repo"):
    if os.path.isdir(_p) and _p not in sys.path:
        sys.path.insert(0, _p)

import numpy as np
import ml_dtypes

import concourse.bacc as bacc
import concourse.mybir as mybir
import concourse.tile as tile
from concourse.bass_utils import run_bass_kernel_spmd
from concourse.masks import make_identity

P = 128
B, D, H, E, KK, V, TOPK = 4096, 1024, 4096, 8, 4096, 50257, 2
NCORES = 8
TOK = B // NCORES            # 512 tokens per core (data-parallel shard)
KD = D // P                  # 8 k-tiles over D
HT = H // P                  # 32 tiles over H
KT = KK // P                 # 32 key tiles
VP = ((V + P - 1) // P) * P  # 50304 padded vocab
VT = VP // P                 # 393 vocab tiles

f32 = mybir.dt.float32
f32r = mybir.dt.float32r
f16 = mybir.dt.float16
# 16-bit compute dtype: fp16 (same PE throughput/footprint as bf16, 8x the
# mantissa; every tensor in this net fits fp16 range comfortably)
bf16 = mybir.dt.float16
bfnp = np.float16

Act = mybir.ActivationFunctionType

_NC_CACHE = {}
LAST_RESULTS = []  # (stage, BassKernelResults) for profiling from test harness


def _chunks(n, step=512):
    out = []
    c0 = 0
    while c0 < n:
        cw = min(step, n - c0)
        out.append((c0, cw))
        c0 += cw
    return out


# ---------------------------------------------------------------- host layout
def _lhst_blocks(w):
    """torch-Linear weight W [out, in] -> lhsT block layout [out/P, P, in/P, P]
    where blk[mt, p, ko, m] = W[mt*P+m, ko*P+p]; blk[mt] DMAs as one
    [P, in/P, P] SBUF tile (contiguous per partition)."""
    o, i = w.shape
    return np.ascontiguousarray(
        w.reshape(o // P, P, i // P, P).transpose(0, 3, 2, 1))


def _featmaj(x):
    """activations [tok, D] -> [P, D/P, tok] feature-major slab."""
    t, d = x.shape
    return np.ascontiguousarray(x.T.reshape(d // P, P, t).transpose(1, 0, 2))


def _bias_tiles(b):
    """bias [n*P] -> [P, n] so column j is the per-partition bias of tile j."""
    return np.ascontiguousarray(b.reshape(-1, P).T)


# ---------------------------------------------------------------- stage 1
def _build_stage1(cap):
    nc = bacc.Bacc("TRN2", target_bir_lowering=False, debug=False,
                   num_devices=NCORES)
    zx = nc.dram_tensor("zx", [P, KD, cap], bf16, kind="ExternalInput")
    w1b = nc.dram_tensor("w1b", [HT, P, KD, P], bf16, kind="ExternalInput")
    b1 = nc.dram_tensor("b1", [P, HT], f32, kind="ExternalInput")
    w2b = nc.dram_tensor("w2b", [KD, P, HT, P], bf16, kind="ExternalInput")
    b2 = nc.dram_tensor("b2", [P, KD], f32, kind="ExternalInput")
    zb = nc.dram_tensor("zb", [P, KD, TOK], f32r, kind="ExternalInput")
    keysb = nc.dram_tensor("keysb", [KT // 4, P, KD, 512], f32r,
                           kind="ExternalInput")
    vb = nc.dram_tensor("vb", [KT, P, D], bf16, kind="ExternalInput")
    yT = nc.dram_tensor("yT", [KD, P, cap], f32, kind="ExternalOutput")
    mT = nc.dram_tensor("mT", [KD, P, TOK], f32, kind="ExternalOutput")

    ch = _chunks(cap)
    with tile.TileContext(nc) as tc:
        with tc.tile_pool(name="const", bufs=1) as const, \
             tc.tile_pool(name="slabs", bufs=1) as slabs, \
             tc.tile_pool(name="wp", bufs=3) as wp, \
             tc.tile_pool(name="w2p", bufs=2) as w2p, \
             tc.tile_pool(name="io", bufs=3) as io, \
             tc.tile_pool(name="kp", bufs=2) as kp, \
             tc.tile_pool(name="ab", bufs=2) as abp, \
             tc.tile_pool(name="ps", bufs=2, space="PSUM") as ps, \
             tc.tile_pool(name="pst", bufs=2, space="PSUM") as pst, \
             tc.tile_pool(name="psa", bufs=4, space="PSUM") as psa:

            ident = const.tile([P, P], bf16)
            make_identity(nc, ident[:])
            b1sb = const.tile([P, HT], f32)
            nc.sync.dma_start(b1sb[:], b1[:])
            b2sb = const.tile([P, KD], f32)
            nc.sync.dma_start(b2sb[:], b2[:])

            # ---- expert MLP on gathered tokens (bf16, fp32 accum) ----
            zxs = slabs.tile([P, KD, cap], bf16, tag="zx")
            nc.sync.dma_start(zxs[:], zx[:])

            for c0, cw in ch:
                hT = slabs.tile([P, HT, 512], bf16, tag="big")
                for ht in range(HT):
                    w1t = wp.tile([P, KD, P], bf16, tag="w1")
                    nc.sync.dma_start(w1t[:], w1b[ht])
                    acc = ps.tile([P, 512], f32, tag="mm")
                    for k in range(KD):
                        nc.tensor.matmul(acc[:, :cw], w1t[:, k, :],
                                         zxs[:, k, c0:c0 + cw],
                                         start=(k == 0), stop=(k == KD - 1))
                    nc.scalar.activation(hT[:, ht, :cw], acc[:, :cw], Act.Relu,
                                         bias=b1sb[:, ht:ht + 1])
                for dt in range(KD):
                    w2t = w2p.tile([P, HT, P], bf16, tag="w2")
                    nc.sync.dma_start(w2t[:], w2b[dt])
                    acc = ps.tile([P, 512], f32, tag="mm")
                    for k in range(HT):
                        nc.tensor.matmul(acc[:, :cw], w2t[:, k, :],
                                         hT[:, k, :cw],
                                         start=(k == 0), stop=(k == HT - 1))
                    yt = io.tile([P, 512], f32, tag="yo")
                    nc.scalar.activation(yt[:, :cw], acc[:, :cw], Act.Identity,
                                         bias=b2sb[:, dt:dt + 1])
                    nc.sync.dma_start(yT[dt][:, c0:c0 + cw], yt[:, :cw])

            # ---- KV memory attention for the B-shard ----
            # scores kept fp32 (softmax exponentiates absolute score error;
            # fp16 storage at |s|~130 would cost ~6% on attention weights).
            # Two passes of 2 token-tiles each so the fp32 slab shares the
            # hT slot (32KB/partition).
            zbs = slabs.tile([P, KD, TOK], f32r, tag="zx")
            nc.sync.dma_start(zbs[:], zb[:])
            NTT = TOK // P  # 4 token tiles
            attnT = slabs.tile([P, KT, P * NTT], bf16, tag="attnT")
            for half in range(2):
                sc = slabs.tile([P, 2, KT // 4, 512], f32, tag="big",
                                name=f"sc{half}")
                for kc in range(KT // 4):  # 8 key chunks of 512
                    kt_ = kp.tile([P, KD, 512], f32r, tag="keys")
                    nc.sync.dma_start(kt_[:], keysb[kc])
                    for th in range(2):
                        t = half * 2 + th
                        acc = ps.tile([P, 512], f32, tag="mm")
                        for k in range(KD):
                            nc.tensor.matmul(acc[:], zbs[:, k, t * P:(t + 1) * P],
                                             kt_[:, k, :],
                                             start=(k == 0), stop=(k == KD - 1))
                        nc.vector.tensor_copy(sc[:, th, kc, :], acc[:])

                for th in range(2):
                    t = half * 2 + th
                    mx = io.tile([P, 1], f32, tag="st")
                    nc.vector.reduce_max(mx[:], sc[:, th],
                                         axis=mybir.AxisListType.XY)
                    nmx = io.tile([P, 1], f32, tag="st")
                    nc.vector.tensor_scalar_mul(nmx[:], mx[:], -1.0)
                    eraw = abp.tile([P, KK], bf16, tag="eraw")
                    se = io.tile([P, 1], f32, tag="st")
                    nc.scalar.activation(eraw[:],
                                         sc[:, th].rearrange("p a b -> p (a b)"),
                                         Act.Exp, bias=nmx[:], accum_out=se[:])
                    rec = io.tile([P, 1], f32, tag="st")
                    nc.vector.reciprocal(rec[:], se[:])
                    ab = abp.tile([P, KK], bf16, tag="ab")
                    nc.vector.tensor_scalar(ab[:], eraw[:], rec[:], None,
                                            mybir.AluOpType.mult)
                    for kt in range(KT):
                        pt = pst.tile([P, P], bf16, tag="tr")
                        nc.tensor.transpose(pt[:], ab[:, kt * P:(kt + 1) * P],
                                            ident[:])
                        nc.vector.tensor_copy(attnT[:, kt, t * P:(t + 1) * P],
                                              pt[:])

            # m.T = values.T @ attn (two groups of 4 d-tiles; 4 psum banks each)
            for g in range(2):
                accs = [psa.tile([P, TOK], f32, tag="acc", name=f"acc{g}_{j}")
                        for j in range(4)]
                for k in range(KT):
                    vt = io.tile([P, D], bf16, tag="vt")
                    nc.sync.dma_start(vt[:], vb[k])
                    for j in range(4):
                        dt = g * 4 + j
                        nc.tensor.matmul(accs[j][:], vt[:, dt * P:(dt + 1) * P],
                                         attnT[:, k, :],
                                         start=(k == 0), stop=(k == KT - 1))
                for j in range(4):
                    ot = io.tile([P, TOK], f32, tag="yo")
                    nc.vector.tensor_copy(ot[:], accs[j][:])
                    nc.sync.dma_start(mT[g * 4 + j], ot[:])

    nc.compile()
    return nc


# ---------------------------------------------------------------- stage 2
def _build_stage2():
    nc = bacc.Bacc("TRN2", target_bir_lowering=False, debug=False,
                   num_devices=NCORES)
    xT = nc.dram_tensor("xT", [P, KD, TOK], f32, kind="ExternalInput")
    gm = {}
    for i in (1, 2):
        gm[i] = dict(
            fc1=nc.dram_tensor(f"m{i}fc1", [HT, P, KD, P], bf16, kind="ExternalInput"),
            b1=nc.dram_tensor(f"m{i}b1", [P, HT], f32, kind="ExternalInput"),
            fc2=nc.dram_tensor(f"m{i}fc2", [KD, P, HT, P], bf16, kind="ExternalInput"),
            b2=nc.dram_tensor(f"m{i}b2", [P, KD], f32, kind="ExternalInput"),
            gw=nc.dram_tensor(f"m{i}gw", [KD, P, KD, P], bf16, kind="ExternalInput"),
            bg=nc.dram_tensor(f"m{i}bg", [P, KD], f32, kind="ExternalInput"),
        )
    adw = nc.dram_tensor("adw", [VT, P, KD, P], bf16, kind="ExternalInput")
    adb = nc.dram_tensor("adb", [P, VT], f32, kind="ExternalInput")
    xoutT = nc.dram_tensor("xoutT", [KD, P, TOK], f32, kind="ExternalOutput")
    logT = nc.dram_tensor("logT", [VT, P, TOK], f32, kind="ExternalOutput")

    with tile.TileContext(nc) as tc:
        with tc.tile_pool(name="const", bufs=1) as const, \
             tc.tile_pool(name="slabs", bufs=1) as slabs, \
             tc.tile_pool(name="wp", bufs=4) as wp, \
             tc.tile_pool(name="w2p", bufs=2) as w2p, \
             tc.tile_pool(name="io", bufs=4) as io, \
             tc.tile_pool(name="ps", bufs=3, space="PSUM") as ps:

            adbsb = const.tile([P, VT], f32)
            nc.sync.dma_start(adbsb[:], adb[:])

            xs = slabs.tile([P, KD, TOK], f32, tag="x")
            nc.sync.dma_start(xs[:], xT[:])

            for i in (1, 2):
                g = gm[i]
                b1sb = const.tile([P, HT], f32, tag=f"b1_{i}")
                nc.sync.dma_start(b1sb[:], g["b1"][:])
                b2sb = const.tile([P, KD], f32, tag=f"b2_{i}")
                nc.sync.dma_start(b2sb[:], g["b2"][:])
                bgsb = const.tile([P, KD], f32, tag=f"bg_{i}")
                nc.sync.dma_start(bgsb[:], g["bg"][:])

                xbf = slabs.tile([P, KD, TOK], bf16, tag=f"xbf_{i}")
                nc.vector.tensor_copy(xbf[:], xs[:])

                h1 = slabs.tile([P, HT, TOK], bf16, tag=f"h1_{i}")
                for ht in range(HT):
                    wt = wp.tile([P, KD, P], bf16, tag="w1")
                    nc.sync.dma_start(wt[:], g["fc1"][ht])
                    acc = ps.tile([P, TOK], f32, tag="mm")
                    for k in range(KD):
                        nc.tensor.matmul(acc[:], wt[:, k, :], xbf[:, k, :],
                                         start=(k == 0), stop=(k == KD - 1))
                    nc.scalar.activation(h1[:, ht, :], acc[:], Act.Gelu,
                                         bias=b1sb[:, ht:ht + 1])
                gt = slabs.tile([P, KD, TOK], f32, tag=f"gt_{i}")
                for dt in range(KD):
                    wt = wp.tile([P, KD, P], bf16, tag="w1")
                    nc.sync.dma_start(wt[:], g["gw"][dt])
                    acc = ps.tile([P, TOK], f32, tag="mm")
                    for k in range(KD):
                        nc.tensor.matmul(acc[:], wt[:, k, :], xbf[:, k, :],
                                         start=(k == 0), stop=(k == KD - 1))
                    nc.scalar.activation(gt[:, dt, :], acc[:], Act.Sigmoid,
                                         bias=bgsb[:, dt:dt + 1])
                for dt in range(KD):
                    wt = w2p.tile([P, HT, P], bf16, tag="w2")
                    nc.sync.dma_start(wt[:], g["fc2"][dt])
                    acc = ps.tile([P, TOK], f32, tag="mm")
                    for k in range(HT):
                        nc.tensor.matmul(acc[:], wt[:, k, :], h1[:, k, :],
                                         start=(k == 0), stop=(k == HT - 1))
                    h2 = io.tile([P, TOK], f32, tag="h2")
                    nc.scalar.activation(h2[:], acc[:], Act.Identity,
                                         bias=b2sb[:, dt:dt + 1])
                    # x += g * h2
                    nc.vector.tensor_mul(h2[:], h2[:], gt[:, dt, :])
                    nc.vector.tensor_add(xs[:, dt, :], xs[:, dt, :], h2[:])

            # final x out
            x2bf = slabs.tile([P, KD, TOK], bf16, tag="x2bf")
            nc.vector.tensor_copy(x2bf[:], xs[:])
            for dt in range(KD):
                nc.sync.dma_start(xoutT[dt], xs[:, dt, :])

            # vocab adapter
            for vt in range(VT):
                wt = wp.tile([P, KD, P], bf16, tag="w1")
                nc.sync.dma_start(wt[:], adw[vt])
                acc = ps.tile([P, TOK], f32, tag="mm")
                for k in range(KD):
                    nc.tensor.matmul(acc[:], wt[:, k, :], x2bf[:, k, :],
                                     start=(k == 0), stop=(k == KD - 1))
                lt = io.tile([P, TOK], f32, tag="lt")
                nc.scalar.activation(lt[:], acc[:], Act.Identity,
                                     bias=adbsb[:, vt:vt + 1])
                nc.sync.dma_start(logT[vt], lt[:])

    nc.compile()
    return nc


# ---------------------------------------------------------------- host driver
def kernel(z, gate_w, gate_b, ew1, eb1, ew2, eb2, keys, values,
           m1_fc1_w, m1_fc1_b, m1_fc2_w, m1_fc2_b, m1_g_w, m1_g_b,
           m2_fc1_w, m2_fc1_b, m2_fc2_w, m2_fc2_b, m2_g_w, m2_g_b,
           ad_w, ad_b):
    global LAST_RESULTS
    LAST_RESULTS = []
    z = np.asarray(z, np.float32)

    # ---- host: fp32 gate + top-2 routing (matches reference numerics) ----
    gl = z @ np.asarray(gate_w, np.float32).T + np.asarray(gate_b, np.float32)
    gl -= gl.max(axis=1, keepdims=True)
    wgate = np.exp(gl)
    wgate /= wgate.sum(axis=1, keepdims=True)
    order = np.argsort(-wgate, axis=1, kind="stable")[:, :TOPK]  # [B, 2]
    topw = np.take_along_axis(wgate, order, axis=1)

    toks = [np.nonzero((order == e).any(axis=1))[0] for e in range(E)]
    wtok = [topw[toks[e]][order[toks[e]] == e] for e in range(E)]
    maxcnt = max(len(t) for t in toks)
    cap = max(512, -(-maxcnt // 256) * 256)

    if ("s1", cap) not in _NC_CACHE:
        _NC_CACHE[("s1", cap)] = _build_stage1(cap)
    nc1 = _NC_CACHE[("s1", cap)]

    zbf = z.astype(bfnp)
    keysb = np.ascontiguousarray(
        keys.reshape(KT // 4, 512, KD, P).transpose(0, 3, 2, 1).astype(np.float32))
    vb = np.ascontiguousarray(values.reshape(KT, P, D).astype(bfnp))

    in_maps1 = []
    for c in range(E):
        zg = np.zeros((cap, D), bfnp)
        zg[:len(toks[c])] = zbf[toks[c]]
        in_maps1.append({
            "zx": _featmaj(zg),
            "w1b": _lhst_blocks(np.asarray(ew1[c], np.float32)).astype(bfnp),
            "b1": _bias_tiles(np.asarray(eb1[c], np.float32)),
            "w2b": _lhst_blocks(np.asarray(ew2[c], np.float32)).astype(bfnp),
            "b2": _bias_tiles(np.asarray(eb2[c], np.float32)),
            "zb": _featmaj(z[c * TOK:(c + 1) * TOK]),
            "keysb": keysb,
            "vb": vb,
        })
    res1 = run_bass_kernel_spmd(nc1, in_maps1, core_ids=list(range(NCORES)))
    LAST_RESULTS.append(("stage1", res1))

    # ---- host combine: x = scatter(topw * y) + m ----
    x = np.zeros((B, D), np.float32)
    for c in range(E):
        y = res1.results[c]["yT"].transpose(2, 0, 1).reshape(cap, D)
        x[toks[c]] += wtok[c][:, None] * y[:len(toks[c])]
    for c in range(NCORES):
        m = res1.results[c]["mT"].transpose(2, 0, 1).reshape(TOK, D)
        x[c * TOK:(c + 1) * TOK] += m

    if ("s2",) not in _NC_CACHE:
        _NC_CACHE[("s2",)] = _build_stage2()
    nc2 = _NC_CACHE[("s2",)]

    shared = {"adw": _lhst_blocks(
        np.concatenate([np.asarray(ad_w, np.float32),
                        np.zeros((VP - V, D), np.float32)]), ).astype(bfnp),
        "adb": _bias_tiles(np.concatenate([np.asarray(ad_b, np.float32),
                                           np.zeros(VP - V, np.float32)]))}
    for i, (f1w, f1b, f2w, f2b, gw_, gb_) in (
            (1, (m1_fc1_w, m1_fc1_b, m1_fc2_w, m1_fc2_b, m1_g_w, m1_g_b)),
            (2, (m2_fc1_w, m2_fc1_b, m2_fc2_w, m2_fc2_b, m2_g_w, m2_g_b))):
        shared[f"m{i}fc1"] = _lhst_blocks(np.asarray(f1w, np.float32)).astype(bfnp)
        shared[f"m{i}b1"] = _bias_tiles(np.asarray(f1b, np.float32))
        shared[f"m{i}fc2"] = _lhst_blocks(np.asarray(f2w, np.float32)).astype(bfnp)
        shared[f"m{i}b2"] = _bias_tiles(np.asarray(f2b, np.float32))
        shared[f"m{i}gw"] = _lhst_blocks(np.asarray(gw_, np.float32)).astype(bfnp)
        shared[f"m{i}bg"] = _bias_tiles(np.asarray(gb_, np.float32))

    in_maps2 = [{"xT": _featmaj(x[c * TOK:(c + 1) * TOK]), **shared}
                for c in range(NCORES)]
    res2 = run_bass_kernel_spmd(nc2, in_maps2, core_ids=list(range(NCORES)))
    LAST_RESULTS.append(("stage2", res2))

    xout = np.concatenate([
        res2.results[c]["xoutT"].transpose(2, 0, 1).reshape(TOK, D)
        for c in range(NCORES)])
    logits = np.concatenate([
        res2.results[c]["logT"].transpose(2, 0, 1).reshape(TOK, VP)[:, :V]
        for c in range(NCORES)])
    return xout, logits


# revision 17
# speedup vs baseline: 1.1108x; 1.1108x over previous
"""Trainium2 Bass kernel for CompositeTransitionNet (MoE + KV-memory + 2x gated MLP + vocab adapter).

Sharding: expert-parallel MoE (1 expert/core, host-side top-2 dispatch/combine),
data-parallel over B for KV attention, gated MLPs and the vocab adapter.
All activations kept feature-major ([feature, token]) on device; every weight is
pre-transposed on the host into LHS-T block layout so no device transposes are
needed outside the attention probs.
"""
import os
import sys

for _p in ("/opt/trn_rl_repo", "/root/.axon_site/_ro/trn_rl_repo"):
    if os.path.isdir(_p) and _p not in sys.path:
        sys.path.insert(0, _p)

import numpy as np
import ml_dtypes

import concourse.bacc as bacc
import concourse.mybir as mybir
import concourse.tile as tile
from concourse.bass_utils import run_bass_kernel_spmd
from concourse.masks import make_identity

P = 128
B, D, H, E, KK, V, TOPK = 4096, 1024, 4096, 8, 4096, 50257, 2
NCORES = 8
TOK = B // NCORES            # 512 tokens per core (data-parallel shard)
KD = D // P                  # 8 k-tiles over D
HT = H // P                  # 32 tiles over H
KT = KK // P                 # 32 key tiles
VP = ((V + P - 1) // P) * P  # 50304 padded vocab
VT = VP // P                 # 393 vocab tiles

f32 = mybir.dt.float32
f32r = mybir.dt.float32r
f16 = mybir.dt.float16
# 16-bit compute dtype: fp16 (same PE throughput/footprint as bf16, 8x the
# mantissa; every tensor in this net fits fp16 range comfortably)
bf16 = mybir.dt.float16
bfnp = np.float16

Act = mybir.ActivationFunctionType

_NC_CACHE = {}
LAST_RESULTS = []  # (stage, BassKernelResults) for profiling from test harness


def _chunks(n, step=512):
    out = []
    c0 = 0
    while c0 < n:
        cw = min(step, n - c0)
        out.append((c0, cw))
        c0 += cw
    return out


# ---------------------------------------------------------------- host layout
def _lhst_blocks(w):
    """torch-Linear weight W [out, in] -> lhsT block layout [out/P, P, in/P, P]
    where blk[mt, p, ko, m] = W[mt*P+m, ko*P+p]; blk[mt] DMAs as one
    [P, in/P, P] SBUF tile (contiguous per partition)."""
    o, i = w.shape
    return np.ascontiguousarray(
        w.reshape(o // P, P, i // P, P).transpose(0, 3, 2, 1))


def _featmaj(x):
    """activations [tok, D] -> [P, D/P, tok] feature-major slab."""
    t, d = x.shape
    return np.ascontiguousarray(x.T.reshape(d // P, P, t).transpose(1, 0, 2))


def _bias_tiles(b):
    """bias [n*P] -> [P, n] so column j is the per-partition bias of tile j."""
    return np.ascontiguousarray(b.reshape(-1, P).T)


# ---------------------------------------------------------------- stage 1
def _build_stage1(cap):
    nc = bacc.Bacc("TRN2", target_bir_lowering=False, debug=False,
                   num_devices=NCORES)
    zx = nc.dram_tensor("zx", [P, KD, cap], bf16, kind="ExternalInput")
    w1b = nc.dram_tensor("w1b", [HT, P, KD, P], bf16, kind="ExternalInput")
    b1 = nc.dram_tensor("b1", [P, HT], f32, kind="ExternalInput")
    w2b = nc.dram_tensor("w2b", [KD, P, HT, P], bf16, kind="ExternalInput")
    b2 = nc.dram_tensor("b2", [P, KD], f32, kind="ExternalInput")
    zb = nc.dram_tensor("zb", [P, KD, TOK], f32r, kind="ExternalInput")
    keysb = nc.dram_tensor("keysb", [KT // 4, P, KD, 512], f32r,
                           kind="ExternalInput")
    vb = nc.dram_tensor("vb", [KT, P, D], bf16, kind="ExternalInput")
    yT = nc.dram_tensor("yT", [KD, P, cap], f32, kind="ExternalOutput")
    mT = nc.dram_tensor("mT", [KD, P, TOK], f32, kind="ExternalOutput")

    ch = _chunks(cap)
    with tile.TileContext(nc) as tc:
        with tc.tile_pool(name="const", bufs=1) as const, \
             tc.tile_pool(name="slabs", bufs=1) as slabs, \
             tc.tile_pool(name="wp", bufs=3) as wp, \
             tc.tile_pool(name="w2p", bufs=2) as w2p, \
             tc.tile_pool(name="io", bufs=3) as io, \
             tc.tile_pool(name="kp", bufs=2) as kp, \
             tc.tile_pool(name="ab", bufs=2) as abp, \
             tc.tile_pool(name="ps", bufs=2, space="PSUM") as ps, \
             tc.tile_pool(name="pst", bufs=2, space="PSUM") as pst, \
             tc.tile_pool(name="psa", bufs=4, space="PSUM") as psa:

            ident = const.tile([P, P], bf16)
            make_identity(nc, ident[:])
            b1sb = const.tile([P, HT], f32)
            nc.sync.dma_start(b1sb[:], b1[:])
            b2sb = const.tile([P, KD], f32)
            nc.sync.dma_start(b2sb[:], b2[:])

            # ---- expert MLP on gathered tokens (fp16, fp32 accum) ----
            zxs = slabs.tile([P, KD, cap], bf16, tag="zx")
            for k in range(KD):  # split so the first matmul can start early
                nc.sync.dma_start(zxs[:, k, :], zx[:, k, :])

            hT = slabs.tile([P, HT, cap], bf16, tag="big")
            for ht in range(HT):
                w1t = wp.tile([P, KD, P], bf16, tag="w1")
                nc.sync.dma_start(w1t[:], w1b[ht])
                for c0, cw in ch:
                    acc = ps.tile([P, 512], f32, tag="mm")
                    for k in range(KD):
                        nc.tensor.matmul(acc[:, :cw], w1t[:, k, :],
                                         zxs[:, k, c0:c0 + cw],
                                         start=(k == 0), stop=(k == KD - 1))
                    nc.scalar.activation(hT[:, ht, c0:c0 + cw], acc[:, :cw],
                                         Act.Relu, bias=b1sb[:, ht:ht + 1])
            # prefetch the first KV keys during the fc2 window so the
            # expert->KV transition has no DMA bubble
            kt_pre = []
            for kc in range(2):
                kt_ = kp.tile([P, KD, 512], f32r, tag="keys", name=f"kpre{kc}")
                nc.sync.dma_start(kt_[:], keysb[kc])
                kt_pre.append(kt_)

            for dt in range(KD):
                w2t = w2p.tile([P, HT, P], bf16, tag="w2")
                nc.sync.dma_start(w2t[:], w2b[dt])
                for c0, cw in ch:
                    acc = ps.tile([P, 512], f32, tag="mm")
                    for k in range(HT):
                        nc.tensor.matmul(acc[:, :cw], w2t[:, k, :],
                                         hT[:, k, c0:c0 + cw],
                                         start=(k == 0), stop=(k == HT - 1))
                    yt = io.tile([P, 512], f32, tag="yo")
                    nc.scalar.activation(yt[:, :cw], acc[:, :cw], Act.Identity,
                                         bias=b2sb[:, dt:dt + 1])
                    nc.sync.dma_start(yT[dt][:, c0:c0 + cw], yt[:, :cw])

            # ---- KV memory attention for the B-shard ----
            # zb reuses the zx slot (expert fc1 is done with zx by then)
            zbs = slabs.tile([P, KD, TOK], f32r, tag="zx")
            nc.sync.dma_start(zbs[:], zb[:])
            NTT = TOK // P  # 4 token tiles
            NKC = KT // 4   # 8 key chunks
            attnT = slabs.tile([P, KT, P * NTT], bf16, tag="attnT")
            # scores stored fp16 but pre-shifted by the chunk max, so fp16
            # resolution near 0 (5e-4) applies where softmax is sensitive
            sc = slabs.tile([P, NTT, NKC, 512], f16, tag="big")
            pmax = const.tile([P, NTT, NKC], f32)
            for kc in range(NKC):
                if kc < 2:
                    kt_ = kt_pre[kc]
                else:
                    kt_ = kp.tile([P, KD, 512], f32r, tag="keys")
                    nc.sync.dma_start(kt_[:], keysb[kc])
                for t in range(NTT):
                    acc = ps.tile([P, 512], f32, tag="mm")
                    for k in range(KD):
                        nc.tensor.matmul(acc[:], zbs[:, k, t * P:(t + 1) * P],
                                         kt_[:, k, :],
                                         start=(k == 0), stop=(k == KD - 1))
                    nc.vector.reduce_max(pmax[:, t, kc:kc + 1], acc[:],
                                         axis=mybir.AxisListType.X)
                    nc.vector.tensor_scalar(sc[:, t, kc, :], acc[:],
                                            pmax[:, t, kc:kc + 1], None,
                                            mybir.AluOpType.subtract)

            for t in range(NTT):
                mx = io.tile([P, 1], f32, tag="st")
                nc.vector.reduce_max(mx[:], pmax[:, t], axis=mybir.AxisListType.X)
                eraw = abp.tile([P, KK], bf16, tag="eraw")
                ses = io.tile([P, NKC], f32, tag="ses")
                for kc in range(NKC):
                    dkc = io.tile([P, 1], f32, tag="st")
                    nc.vector.tensor_tensor(dkc[:], pmax[:, t, kc:kc + 1], mx[:],
                                            mybir.AluOpType.subtract)
                    nc.scalar.activation(eraw[:, kc * 512:(kc + 1) * 512],
                                         sc[:, t, kc, :], Act.Exp,
                                         bias=dkc[:],
                                         accum_out=ses[:, kc:kc + 1])
                se = io.tile([P, 1], f32, tag="st")
                nc.vector.reduce_sum(se[:], ses[:], axis=mybir.AxisListType.X)
                rec = io.tile([P, 1], f32, tag="st")
                nc.vector.reciprocal(rec[:], se[:])
                # diag(1/sum): the "transpose" matmul rescales columns for free
                dg = io.tile([P, P], bf16, tag="dg")
                nc.vector.tensor_scalar(dg[:], ident[:], rec[:], None,
                                        mybir.AluOpType.mult)
                for kt in range(KT):
                    pt = pst.tile([P, P], f32, tag="tr")
                    nc.tensor.matmul(pt[:], eraw[:, kt * P:(kt + 1) * P], dg[:],
                                     start=True, stop=True)
                    nc.vector.tensor_copy(attnT[:, kt, t * P:(t + 1) * P],
                                          pt[:])

            # m.T = values.T @ attn (two groups of 4 d-tiles; 4 psum banks
            # each). rhs is split per token-tile so these matmuls can start
            # as soon as each token-tile's transposes land, overlapping the
            # remaining softmax work.
            for g in range(2):
                accs = [psa.tile([P, TOK], f32, tag="acc", name=f"acc{g}_{j}")
                        for j in range(4)]
                for k in range(KT):
                    vt = io.tile([P, D], bf16, tag="vt", bufs=6)
                    nc.sync.dma_start(vt[:], vb[k])
                    for j in range(4):
                        dt = g * 4 + j
                        for t in range(NTT):
                            # start=True clears the WHOLE bank, so only the
                            # first write of each bank may set it; later
                            # token-tiles rely on per-element has_written
                            nc.tensor.matmul(
                                accs[j][:, t * P:(t + 1) * P],
                                vt[:, dt * P:(dt + 1) * P],
                                attnT[:, k, t * P:(t + 1) * P],
                                start=(k == 0 and t == 0),
                                stop=(k == KT - 1 and t == NTT - 1),
                                skip_group_check=True)
                for j in range(4):
                    ot = io.tile([P, TOK], f32, tag="yo")
                    nc.vector.tensor_copy(ot[:], accs[j][:])
                    nc.sync.dma_start(mT[g * 4 + j], ot[:])

    nc.compile()
    return nc


# ---------------------------------------------------------------- stage 2
def _build_stage2():
    nc = bacc.Bacc("TRN2", target_bir_lowering=False, debug=False,
                   num_devices=NCORES)
    xT = nc.dram_tensor("xT", [P, KD, TOK], f32, kind="ExternalInput")
    gm = {}
    for i in (1, 2):
        gm[i] = dict(
            fc1=nc.dram_tensor(f"m{i}fc1", [HT, P, KD, P], bf16, kind="ExternalInput"),
            b1=nc.dram_tensor(f"m{i}b1", [P, HT], f32, kind="ExternalInput"),
            fc2=nc.dram_tensor(f"m{i}fc2", [KD, P, HT, P], bf16, kind="ExternalInput"),
            b2=nc.dram_tensor(f"m{i}b2", [P, KD], f32, kind="ExternalInput"),
            gw=nc.dram_tensor(f"m{i}gw", [KD, P, KD, P], bf16, kind="ExternalInput"),
            bg=nc.dram_tensor(f"m{i}bg", [P, KD], f32, kind="ExternalInput"),
        )
    adw = nc.dram_tensor("adw", [VT, P, KD, P], bf16, kind="ExternalInput")
    adb = nc.dram_tensor("adb", [P, VT], f32, kind="ExternalInput")
    xoutT = nc.dram_tensor("xoutT", [KD, P, TOK], f32, kind="ExternalOutput")
    logT = nc.dram_tensor("logT", [VT, P, TOK], f32, kind="ExternalOutput")

    with tile.TileContext(nc) as tc:
        with tc.tile_pool(name="const", bufs=1) as const, \
             tc.tile_pool(name="slabs", bufs=1) as slabs, \
             tc.tile_pool(name="wp", bufs=4) as wp, \
             tc.tile_pool(name="w2p", bufs=2) as w2p, \
             tc.tile_pool(name="io", bufs=4) as io, \
             tc.tile_pool(name="ps", bufs=3, space="PSUM") as ps:

            adbsb = const.tile([P, VT], f32)
            nc.sync.dma_start(adbsb[:], adb[:])

            xs = slabs.tile([P, KD, TOK], f32, tag="x")
            nc.sync.dma_start(xs[:], xT[:])

            for i in (1, 2):
                g = gm[i]
                b1sb = const.tile([P, HT], f32, tag=f"b1_{i}")
                nc.sync.dma_start(b1sb[:], g["b1"][:])
                b2sb = const.tile([P, KD], f32, tag=f"b2_{i}")
                nc.sync.dma_start(b2sb[:], g["b2"][:])
                bgsb = const.tile([P, KD], f32, tag=f"bg_{i}")
                nc.sync.dma_start(bgsb[:], g["bg"][:])

                xbf = slabs.tile([P, KD, TOK], bf16, tag=f"xbf_{i}")
                nc.vector.tensor_copy(xbf[:], xs[:])

                h1 = slabs.tile([P, HT, TOK], bf16, tag=f"h1_{i}")
                for ht in range(HT):
                    wt = wp.tile([P, KD, P], bf16, tag="w1")
                    nc.sync.dma_start(wt[:], g["fc1"][ht])
                    acc = ps.tile([P, TOK], f32, tag="mm")
                    for k in range(KD):
                        nc.tensor.matmul(acc[:], wt[:, k, :], xbf[:, k, :],
                                         start=(k == 0), stop=(k == KD - 1))
                    nc.scalar.activation(h1[:, ht, :], acc[:], Act.Gelu,
                                         bias=b1sb[:, ht:ht + 1])
                gt = slabs.tile([P, KD, TOK], f32, tag=f"gt_{i}")
                for dt in range(KD):
                    wt = wp.tile([P, KD, P], bf16, tag="w1")
                    nc.sync.dma_start(wt[:], g["gw"][dt])
                    acc = ps.tile([P, TOK], f32, tag="mm")
                    for k in range(KD):
                        nc.tensor.matmul(acc[:], wt[:, k, :], xbf[:, k, :],
                                         start=(k == 0), stop=(k == KD - 1))
                    nc.scalar.activation(gt[:, dt, :], acc[:], Act.Sigmoid,
                                         bias=bgsb[:, dt:dt + 1])
                for dt in range(KD):
                    wt = w2p.tile([P, HT, P], bf16, tag="w2")
                    nc.sync.dma_start(wt[:], g["fc2"][dt])
                    acc = ps.tile([P, TOK], f32, tag="mm")
                    for k in range(HT):
                        nc.tensor.matmul(acc[:], wt[:, k, :], h1[:, k, :],
                                         start=(k == 0), stop=(k == HT - 1))
                    h2 = io.tile([P, TOK], f32, tag="h2")
                    nc.scalar.activation(h2[:], acc[:], Act.Identity,
                                         bias=b2sb[:, dt:dt + 1])
                    # x += g * h2
                    nc.vector.tensor_mul(h2[:], h2[:], gt[:, dt, :])
                    nc.vector.tensor_add(xs[:, dt, :], xs[:, dt, :], h2[:])

            # final x out
            x2bf = slabs.tile([P, KD, TOK], bf16, tag="x2bf")
            nc.vector.tensor_copy(x2bf[:], xs[:])
            for dt in range(KD):
                nc.sync.dma_start(xoutT[dt], xs[:, dt, :])

            # vocab adapter
            for vt in range(VT):
                wt = wp.tile([P, KD, P], bf16, tag="w1")
                nc.sync.dma_start(wt[:], adw[vt])
                acc = ps.tile([P, TOK], f32, tag="mm")
                for k in range(KD):
                    nc.tensor.matmul(acc[:], wt[:, k, :], x2bf[:, k, :],
                                     start=(k == 0), stop=(k == KD - 1))
                lt = io.tile([P, TOK], f32, tag="lt")
                nc.scalar.activation(lt[:], acc[:], Act.Identity,
                                     bias=adbsb[:, vt:vt + 1])
                nc.sync.dma_start(logT[vt], lt[:])

    nc.compile()
    return nc


# ---------------------------------------------------------------- host driver
def kernel(z, gate_w, gate_b, ew1, eb1, ew2, eb2, keys, values,
           m1_fc1_w, m1_fc1_b, m1_fc2_w, m1_fc2_b, m1_g_w, m1_g_b,
           m2_fc1_w, m2_fc1_b, m2_fc2_w, m2_fc2_b, m2_g_w, m2_g_b,
           ad_w, ad_b):
    global LAST_RESULTS
    LAST_RESULTS = []
    z = np.asarray(z, np.float32)

    # ---- host: fp32 gate + top-2 routing (matches reference numerics) ----
    gl = z @ np.asarray(gate_w, np.float32).T + np.asarray(gate_b, np.float32)
    gl -= gl.max(axis=1, keepdims=True)
    wgate = np.exp(gl)
    wgate /= wgate.sum(axis=1, keepdims=True)
    order = np.argsort(-wgate, axis=1, kind="stable")[:, :TOPK]  # [B, 2]
    topw = np.take_along_axis(wgate, order, axis=1)

    toks = [np.nonzero((order == e).any(axis=1))[0] for e in range(E)]
    wtok = [topw[toks[e]][order[toks[e]] == e] for e in range(E)]
    maxcnt = max(len(t) for t in toks)
    cap = max(512, -(-maxcnt // 64) * 64)

    if ("s1", cap) not in _NC_CACHE:
        _NC_CACHE[("s1", cap)] = _build_stage1(cap)
    nc1 = _NC_CACHE[("s1", cap)]

    zbf = z.astype(bfnp)
    keysb = np.ascontiguousarray(
        keys.reshape(KT // 4, 512, KD, P).transpose(0, 3, 2, 1).astype(np.float32))
    vb = np.ascontiguousarray(values.reshape(KT, P, D).astype(bfnp))

    in_maps1 = []
    for c in range(E):
        zg = np.zeros((cap, D), bfnp)
        zg[:len(toks[c])] = zbf[toks[c]]
        in_maps1.append({
            "zx": _featmaj(zg),
            "w1b": _lhst_blocks(np.asarray(ew1[c], np.float32)).astype(bfnp),
            "b1": _bias_tiles(np.asarray(eb1[c], np.float32)),
            "w2b": _lhst_blocks(np.asarray(ew2[c], np.float32)).astype(bfnp),
            "b2": _bias_tiles(np.asarray(eb2[c], np.float32)),
            "zb": _featmaj(z[c * TOK:(c + 1) * TOK]),
            "keysb": keysb,
            "vb": vb,
        })
    res1 = run_bass_kernel_spmd(nc1, in_maps1, core_ids=list(range(NCORES)))
    LAST_RESULTS.append(("stage1", res1))

    # ---- host combine: x = scatter(topw * y) + m ----
    x = np.zeros((B, D), np.float32)
    for c in range(E):
        y = res1.results[c]["yT"].transpose(2, 0, 1).reshape(cap, D)
        x[toks[c]] += wtok[c][:, None] * y[:len(toks[c])]
    for c in range(NCORES):
        m = res1.results[c]["mT"].transpose(2, 0, 1).reshape(TOK, D)
        x[c * TOK:(c + 1) * TOK] += m

    if ("s2",) not in _NC_CACHE:
        _NC_CACHE[("s2",)] = _build_stage2()
    nc2 = _NC_CACHE[("s2",)]

    shared = {"adw": _lhst_blocks(
        np.concatenate([np.asarray(ad_w, np.float32),
                        np.zeros((VP - V, D), np.float32)]), ).astype(bfnp),
        "adb": _bias_tiles(np.concatenate([np.asarray(ad_b, np.float32),
                                           np.zeros(VP - V, np.float32)]))}
    for i, (f1w, f1b, f2w, f2b, gw_, gb_) in (
            (1, (m1_fc1_w, m1_fc1_b, m1_fc2_w, m1_fc2_b, m1_g_w, m1_g_b)),
            (2, (m2_fc1_w, m2_fc1_b, m2_fc2_w, m2_fc2_b, m2_g_w, m2_g_b))):
        shared[f"m{i}fc1"] = _lhst_blocks(np.asarray(f1w, np.float32)).astype(bfnp)
        shared[f"m{i}b1"] = _bias_tiles(np.asarray(f1b, np.float32))
        shared[f"m{i}fc2"] = _lhst_blocks(np.asarray(f2w, np.float32)).astype(bfnp)
        shared[f"m{i}b2"] = _bias_tiles(np.asarray(f2b, np.float32))
        shared[f"m{i}gw"] = _lhst_blocks(np.asarray(gw_, np.float32)).astype(bfnp)
        shared[f"m{i}bg"] = _bias_tiles(np.asarray(gb_, np.float32))

    in_maps2 = [{"xT": _featmaj(x[c * TOK:(c + 1) * TOK]), **shared}
                for c in range(NCORES)]
    res2 = run_bass_kernel_spmd(nc2, in_maps2, core_ids=list(range(NCORES)))
    LAST_RESULTS.append(("stage2", res2))

    xout = np.concatenate([
        res2.results[c]["xoutT"].transpose(2, 0, 1).reshape(TOK, D)
        for c in range(NCORES)])
    logits = np.concatenate([
        res2.results[c]["logT"].transpose(2, 0, 1).reshape(TOK, VP)[:, :V]
        for c in range(NCORES)])
    return xout, logits
